# revision 18
# baseline (speedup 1.0000x reference)
"""Child-Sum TreeLSTM over a perfect binary tree (N=65535, depth 15) on 8 trn2 cores.

Sharding: each core owns one depth-3 subtree (levels 15..3 are fully local:
children of node range [a,b) are [2a+1,2b+1), so per-core level slices are
contiguous and child gathers are stride-2 local reads).  The 8 subtree roots
are AllGathered (4 KB) and the top 7 nodes are computed redundantly on every
core; the host takes them from core 0.

On-chip layout is feature-major ([feature-chunk=128 partitions, nodes free]);
the host pre-transposes the inputs so the device never transposes anything.
Biases are folded into the x-side matmul via an appended ones-row (K=301).
Matmuls run in bf16 (fp32 PSUM accumulation); cell state is fp32.

PSUM discipline: every accumulation group owns one full 2KB bank (the zero
region): psum tiles are [128, 2, 512] f32 (2 banks), 4 pool slots = 8 banks.
Matmuls are emitted chunk-major (x- and h-contributions of one 128-feature
chunk back to back) so accumulation groups close early and the sigma reads
free psum slots while the block is still running.  The f-gate h-side matmuls
(fpre, computed at the child level) depend on that block's slow gate chain,
so they are deferred by one block - emitted after the next block's dense
matmul run, by which time the chain has drained and the PE does not stall.
The x-side of the tiny levels (S<=128) plus the 7 top nodes is batched into
one N=134 matmul pass up front so the serial tree tail only runs h-side work.
"""

import sys

sys.path.insert(0, "/opt/trn_rl_repo")

import numpy as np
import ml_dtypes

IN_DIM = 300
MEM = 256
DEPTH = 15
N_NODES = 2 ** (DEPTH + 1) - 1  # 65535
NCORES = 8
SUB_DEPTH = 3  # shard at depth 3 -> 8 subtrees
NB = 512  # node block size (one psum bank per 128-feature chunk)
SMALL_S = 128  # levels with S <= this use the batched x pass
# K padded 301 -> 384: matmuls with <128-partition weights run at half the
# sustained PE rate (no FWL; hw-measured 427 vs 216 ns per 512-col MM), so
# the partial 45-row chunk is padded with zero weight rows to keep the whole
# stream warm.
K_PAD = 384

LEVELS = list(range(DEPTH, SUB_DEPTH - 1, -1))  # 15..3
S_OF = {d: 2 ** (d - SUB_DEPTH) for d in LEVELS}  # 4096..1
OFF_OF = {}
_off = 0
for _d in LEVELS:
    OFF_OF[_d] = _off
    _off += S_OF[_d]
N_LOCAL = _off  # 8191
TOP_COL0 = N_LOCAL  # columns 8191..8197 hold x of global nodes 0..6
N_COLS = N_LOCAL + 7  # 8198
_batch_levels = [d for d in LEVELS if S_OF[d] <= SMALL_S]
XSB_COL0 = OFF_OF[_batch_levels[0]]  # first column served by the batched x pass
XSB_N = N_COLS - XSB_COL0  # 134

_CACHE = {}


def _ceil_div(a, b):
    return -(-a // b)


def _build_program():
    import concourse.mybir as mybir
    import concourse.bacc as bacc
    from concourse import tile

    f32 = mybir.dt.float32
    bf16 = mybir.dt.bfloat16
    f8 = mybir.dt.float8e4
    DR = mybir.MatmulPerfMode.DoubleRow
    SIG = mybir.ActivationFunctionType.Sigmoid
    TANH = mybir.ActivationFunctionType.Tanh

    nc = bacc.Bacc("TRN2", target_bir_lowering=False, debug=False, num_devices=NCORES)

    # NOTE: fp8 DoubleRow for the x-side was tried and reverted: the HAM
    # power-throttles the PE to 4/8 clock while fp8 runs (min matmul dur
    # 426ns = 2x the bf16 floor), erasing the throughput gain for the whole
    # stream.
    xt = nc.dram_tensor("xt", [K_PAD, N_COLS], bf16, kind="ExternalInput")
    wx = nc.dram_tensor("wx", [K_PAD, 4 * MEM], bf16, kind="ExternalInput")
    wh = nc.dram_tensor("wh", [MEM, 3 * MEM], bf16, kind="ExternalInput")
    wf = nc.dram_tensor("wf", [MEM, MEM], bf16, kind="ExternalInput")
    out = nc.dram_tensor("out", [2, 128, N_COLS], f32, kind="ExternalOutput")
    # h written in its native bf16 (the host upcasts): 25% less output DMA
    outh = nc.dram_tensor("outh", [2, 128, N_COLS], bf16, kind="ExternalOutput")

    KCH = [(0, 128), (128, 128), (256, 128)]  # k chunks of K_PAD=384

    with tile.TileContext(nc) as tc:
        with (
            tc.tile_pool(name="const", bufs=1) as cpool,
            tc.tile_pool(name="perst", bufs=1) as ppool,
            tc.tile_pool(name="xp", bufs=3) as xpool,
            tc.tile_pool(name="wk", bufs=2) as wk,
            tc.tile_pool(name="ps", bufs=4, space="PSUM") as psp,
            tc.tile_pool(name="dram", bufs=1, space="DRAM") as dram,
        ):
            # ---- PE warm-up: dense junk matmuls with no input deps run
            # during the initial DMA window so the HAM un-throttles the PE
            # clock (4/8 -> 8/8) before the real matmul stream begins
            jw = wk.tile([128, 128], bf16, tag="jw", name="jw", bufs=1)
            jx = wk.tile([128, NB], bf16, tag="jx", name="jx", bufs=1)
            nc.vector.memset(jw[:], 0.0)
            nc.vector.memset(jx[:], 0.0)
            pw = [
                psp.tile([128, 2, NB], f32, tag="ps", name=f"pw{j}") for j in range(2)
            ]
            # 40 reps: enough to bridge the first x-DMA window so the HAM
            # does not re-throttle to 4/8 before the real stream begins
            for i in range(40):
                nc.tensor.matmul(
                    pw[(i // 2) % 2][:, i % 2, :],
                    jw[:],
                    jx[:],
                    start=True,
                    stop=True,
                )
            # consume the warm-up results so they are not dead-code-eliminated
            # (gin is fully overwritten by the real root DMAs later)
            jo = wk.tile([128, 2], f32, tag="jo", name="jo", bufs=1)
            nc.vector.tensor_copy(jo[:], pw[0][:, 0, 0:2])
            nc.vector.tensor_copy(jo[:], pw[1][:, 1, 0:2])

            # ---- load weights ----
            wx_sb = []
            for i, (k0, kn) in enumerate(KCH):
                t = cpool.tile([kn, 4 * MEM], bf16, tag=f"wx{i}", name=f"wx{i}")
                nc.sync.dma_start(t[:], wx[k0 : k0 + kn, :])
                wx_sb.append(t)
            wh_sb = []
            for i in range(2):
                t = cpool.tile([128, 3 * MEM], bf16, tag=f"wh{i}", name=f"wh{i}")
                nc.sync.dma_start(t[:], wh[i * 128 : (i + 1) * 128, :])
                wh_sb.append(t)
            wf_sb = []
            for i in range(2):
                t = cpool.tile([128, MEM], bf16, tag=f"wf{i}", name=f"wf{i}")
                nc.sync.dma_start(t[:], wf[i * 128 : (i + 1) * 128, :])
                wf_sb.append(t)

            # ---- warm-up collective: a junk 1KB AllGather issued up front
            # keeps ncfw/the CC stream warm so the real root gather at the
            # end does not pay the cold-start trigger latency.
            gin_w = dram.tile([32, 1], f32)
            nc.sync.dma_start(gin_w[:, :], jo[0:32, 0:1])
            gout_w = nc.dram_tensor(
                "gout_w", [32 * NCORES, 1], f32, kind="Internal", addr_space="Shared"
            )
            nc.gpsimd.collective_compute(
                "AllGather",
                mybir.AluOpType.bypass,
                replica_groups=[list(range(NCORES))],
                ins=[gin_w.opt()],
                outs=[gout_w[:, :].opt()],
            )

            # ---- persistent level buffers (A = odd levels, B = even) ----
            hbuf = {
                1: ppool.tile([128, 2, 4096], bf16, tag="hA", name="hA"),
                0: ppool.tile([128, 2, 2048], bf16, tag="hB", name="hB"),
            }
            cbuf = {
                1: ppool.tile([128, 2, 4096], f32, tag="cA", name="cA"),
                0: ppool.tile([128, 2, 2048], f32, tag="cB", name="cB"),
            }
            fbuf = {
                1: ppool.tile([128, 2, 4096], bf16, tag="fA", name="fA"),
                0: ppool.tile([128, 2, 2048], bf16, tag="fB", name="fB"),
            }

            def load_x(col0, s):
                ts_ = []
                for i, (k0, kn) in enumerate(KCH):
                    t = xpool.tile([kn, NB], bf16, tag=f"xk{i}", name=f"xk{i}")
                    nc.sync.dma_start(t[:, :s], xt[k0 : k0 + kn, col0 : col0 + s])
                    ts_.append(t)
                return ts_


            # ---- persistent x tiles for the tiny levels + top (262 cols) ----
            xs_small = []
            for i, (k0, kn) in enumerate(KCH):
                t = cpool.tile([kn, XSB_N], bf16, tag=f"xs{i}", name=f"xs{i}")
                nc.sync.dma_start(t[:], xt[k0 : k0 + kn, XSB_COL0:N_COLS])
                xs_small.append(t)

            def do_level(
                S,
                x_col0,
                out_col0,
                h_child,  # bf16 AP [128, 2, 2S] or None for leaves
                c_child,  # f32 AP [128, 2, 2S] or None
                f_child,  # bf16 AP [128, 2, 2S] (fpre of children) or None
                h_dest,  # bf16 AP [128, 2, >=S]
                c_dest,  # f32 AP [128, 2, >=S]
                fpre_out,  # bf16 AP [128, 2, >=S] or None
                root_sink=None,  # gin dram tile, for S==1 (subtree root)
                pending=None,  # deferred fpre thunks (shared across levels)
            ):
                leaf = h_child is None
                for b in range(_ceil_div(S, NB)):
                    col = b * NB
                    s = min(NB, S - col)
                    n_m = 6 if leaf else 8
                    hs = None
                    if not leaf:
                        # child-h sum on gpsimd: DVE is the binding engine at
                        # warm PE rate, Pool has slack
                        hs = wk.tile([128, 2, NB], bf16, tag="hs", name="hs")
                        nc.gpsimd.tensor_add(
                            hs[:, :, :s],
                            h_child[:, :, 2 * col : 2 * (col + s) : 2],
                            h_child[:, :, 2 * col + 1 : 2 * (col + s) : 2],
                        )
                    if True:
                        xts = load_x(x_col0 + col, s)
                        pt = [
                            psp.tile([128, 2, NB], f32, tag="ps", name=f"pt{j}")
                            for j in range(n_m // 2)
                        ]
                        # chunk-major: each chunk's accumulation (x then h)
                        # closes before the next chunk starts
                        for m in range(n_m):
                            msl = slice(m * 128, (m + 1) * 128)
                            for ki in range(3):
                                nc.tensor.matmul(
                                    pt[m // 2][:, m % 2, :s],
                                    wx_sb[ki][:, msl],
                                    xts[ki][:, :s],
                                    start=(ki == 0),
                                    stop=(ki == 2 and (leaf or m >= 6)),
                                )
                            if not leaf and m < 6:
                                for hc in range(2):
                                    nc.tensor.matmul(
                                        pt[m // 2][:, m % 2, :s],
                                        wh_sb[hc][:, msl],
                                        hs[:, hc, :s],
                                        start=False,
                                        stop=(hc == 1),
                                    )
                    # flush fpre matmuls deferred from the previous block /
                    # level: by now their gate chains have drained, and the
                    # matmuls above kept the PE stream dense in the meantime
                    for th in pending:
                        th()
                    pending.clear()
                    if not leaf:
                        pre_f = wk.tile([128, 4, NB], bf16, tag="pre_f", name="pre_f")
                        fx_ap = pt[3][:, :, :s]
                        for side in range(2):
                            nc.vector.tensor_add(
                                pre_f[:, 2 * side : 2 * side + 2, :s],
                                f_child[:, :, 2 * col + side : 2 * (col + s) : 2],
                                fx_ap,
                            )
                    # ---- gates ----
                    sig_io = wk.tile([128, 4, NB], f32, tag="sig_io", name="sig_io")
                    u_t = wk.tile([128, 2, NB], f32, tag="u_t", name="u_t")
                    nc.scalar.activation(sig_io[:, 0:2, :s], pt[0][:, :, :s], SIG)
                    nc.scalar.activation(sig_io[:, 2:4, :s], pt[1][:, :, :s], SIG)
                    nc.scalar.activation(u_t[:, :, :s], pt[2][:, :, :s], TANH)
                    cs = c_dest[:, :, col : col + s]
                    nc.vector.tensor_mul(cs, sig_io[:, 0:2, :s], u_t[:, :, :s])
                    if not leaf:
                        sig_f = wk.tile(
                            [128, 4, NB], f32, tag="sig_f", name="sig_f", bufs=1
                        )
                        nc.scalar.activation(sig_f[:, :, :s], pre_f[:, :, :s], SIG)
                        fc = wk.tile([128, 2, NB], f32, tag="fc", name="fc")
                        nc.vector.tensor_mul(
                            fc[:, :, :s],
                            sig_f[:, 0:2, :s],
                            c_child[:, :, 2 * col : 2 * (col + s) : 2],
                        )
                        nc.vector.tensor_add(cs, cs, fc[:, :, :s])
                        # right-child fc chain on gpsimd (engine balancing)
                        fc2 = wk.tile([128, 2, NB], f32, tag="fc", name="fc2")
                        nc.gpsimd.tensor_mul(
                            fc2[:, :, :s],
                            sig_f[:, 2:4, :s],
                            c_child[:, :, 2 * col + 1 : 2 * (col + s) : 2],
                        )
                        nc.gpsimd.tensor_add(cs, cs, fc2[:, :, :s])
                    tc_t = wk.tile([128, 2, NB], f32, tag="tc_t", name="tc_t", bufs=1)
                    nc.scalar.activation(tc_t[:, :, :s], cs, TANH)
                    nc.vector.tensor_mul(
                        h_dest[:, :, col : col + s],
                        sig_io[:, 2:4, :s],
                        tc_t[:, :, :s],
                    )
                    # fpre for this block: deferred until the parent needs it
                    if fpre_out is not None:

                        def fpre_thunk(col=col, s=s, leaf=leaf):
                            psf = psp.tile([128, 2, NB], f32, tag="ps", name="psf")
                            for m in range(2):
                                for hc in range(2):
                                    nc.tensor.matmul(
                                        psf[:, m, :s],
                                        wf_sb[hc][:, m * 128 : (m + 1) * 128],
                                        h_dest[:, hc, col : col + s],
                                        start=(hc == 0),
                                        stop=(hc == 1),
                                    )
                            # gpsimd cannot read PSUM: leaf thunks drain on
                            # the DVE, the rest on ScalarE
                            if leaf:
                                nc.vector.tensor_copy(
                                    fpre_out[:, :, col : col + s], psf[:, :, :s]
                                )
                            else:
                                nc.scalar.copy(
                                    fpre_out[:, :, col : col + s], psf[:, :, :s]
                                )

                        pending.append(fpre_thunk)
                    if root_sink is not None and S == 1:
                        gin = root_sink
                        for ch in range(2):
                            nc.sync.dma_start(
                                gin[ch * 128 : (ch + 1) * 128, 0:1], cs[:, ch, :]
                            )
                            nc.gpsimd.dma_start(
                                gin[ch * 128 : (ch + 1) * 128, 1:2],
                                h_dest[:, ch, col : col + 1],
                            )
                # outputs for the whole level -- on gpsimd so the sync queue
                # only carries x prefetches (an out DMA waits on the level's
                # last gate chain and would stall the next level's x loads)
                for ch in range(2):
                    nc.gpsimd.dma_start(
                        out[ch, :, out_col0 : out_col0 + S], c_dest[:, ch, :S]
                    )
                    nc.gpsimd.dma_start(
                        outh[ch, :, out_col0 : out_col0 + S], h_dest[:, ch, :S]
                    )
                return pending

            def do_small_level(
                S,
                x_col0,
                out_col0,
                h_child,  # bf16 AP [128, 2, 2S]
                c_child,  # f32 AP [128, 2, 2S]
                h_dest,  # bf16 AP [128, 2, >=S]
                c_dest,  # f32 AP [128, 2, >=S]
                root_sink=None,
            ):
                # Single-block level (S <= 128).  The x-side preactivations
                # accumulate directly in PSUM (emitted early, no input deps,
                # so the PE does them during the previous level's gate chain);
                # the h-side matmuls land on top with start=False and the
                # activations then read PSUM directly -- no DVE pre-adds, no
                # hs sum, no fpre round trip.  Layout: ps_io bank=m%2 offset
                # (m//2)*s -> i at [:, :, 0:s], o at [:, :, s:2s], u at 2s:3s;
                # ps_f bank=f-chunk, offset side*s.
                s = S
                xo = x_col0 - XSB_COL0
                ps_io = psp.tile([128, 2, NB], f32, tag="ps", name="ps_io")
                ps_f = psp.tile([128, 2, NB], f32, tag="ps", name="ps_f")
                # start=True marks the WHOLE 2KB bank pending-zero, so each
                # bank gets exactly one start (first matmul in) and one stop
                # (last matmul in); the three chunk regions per bank share it
                for b in range(2):
                    for mi, m in enumerate((b, b + 2, b + 4)):
                        ap = ps_io[:, b, mi * s : (mi + 1) * s]
                        msl = slice(m * 128, (m + 1) * 128)
                        for ki in range(3):
                            nc.tensor.matmul(
                                ap,
                                wx_sb[ki][:, msl],
                                xs_small[ki][:, xo : xo + s],
                                start=(mi == 0 and ki == 0),
                                stop=False,
                            )
                for m in range(2):  # f chunks: one fx copy per child side
                    msl = slice((6 + m) * 128, (7 + m) * 128)
                    for side in range(2):
                        ap = ps_f[:, m, side * s : (side + 1) * s]
                        for ki in range(3):
                            nc.tensor.matmul(
                                ap,
                                wx_sb[ki][:, msl],
                                xs_small[ki][:, xo : xo + s],
                                start=(side == 0 and ki == 0),
                                stop=False,
                            )
                # h-side: children read in place (stride 2), no hs add
                for b in range(2):
                    for mi, m in enumerate((b, b + 2, b + 4)):
                        ap = ps_io[:, b, mi * s : (mi + 1) * s]
                        msl = slice(m * 128, (m + 1) * 128)
                        for side in range(2):
                            for hc in range(2):
                                nc.tensor.matmul(
                                    ap,
                                    wh_sb[hc][:, msl],
                                    h_child[:, hc, side : 2 * s : 2],
                                    start=False,
                                    stop=(mi == 2 and side == 1 and hc == 1),
                                )
                for m in range(2):
                    for side in range(2):
                        ap = ps_f[:, m, side * s : (side + 1) * s]
                        for hc in range(2):
                            nc.tensor.matmul(
                                ap,
                                wf_sb[hc][:, m * 128 : (m + 1) * 128],
                                h_child[:, hc, side : 2 * s : 2],
                                start=False,
                                stop=(side == 1 and hc == 1),
                            )
                # ---- gates (activations read PSUM directly) ----
                sig_io = wk.tile([128, 2, 256], f32, tag="sio_s", name="sio_s")
                u_t = wk.tile([128, 2, 128], f32, tag="u_s", name="u_s")
                sig_f = wk.tile([128, 2, 256], f32, tag="sf_s", name="sf_s")
                nc.scalar.activation(sig_io[:, :, : 2 * s], ps_io[:, :, : 2 * s], SIG)
                nc.scalar.activation(u_t[:, :, :s], ps_io[:, :, 2 * s : 3 * s], TANH)
                nc.scalar.activation(sig_f[:, :, : 2 * s], ps_f[:, :, : 2 * s], SIG)
                cs = c_dest[:, :, 0:s]
                nc.vector.tensor_mul(cs, sig_io[:, :, 0:s], u_t[:, :, :s])
                fc = wk.tile([128, 2, 128], f32, tag="fc_s", name="fc_s")
                nc.vector.tensor_mul(
                    fc[:, :, :s], sig_f[:, :, 0:s], c_child[:, :, 0 : 2 * s : 2]
                )
                nc.vector.tensor_add(cs, cs, fc[:, :, :s])
                fc2 = wk.tile([128, 2, 128], f32, tag="fc_s", name="fc2_s")
                nc.vector.tensor_mul(
                    fc2[:, :, :s], sig_f[:, :, s : 2 * s], c_child[:, :, 1 : 2 * s : 2]
                )
                nc.vector.tensor_add(cs, cs, fc2[:, :, :s])
                tc_t = wk.tile([128, 2, 128], f32, tag="tc_s", name="tc_s", bufs=1)
                nc.scalar.activation(tc_t[:, :, :s], cs, TANH)
                nc.vector.tensor_mul(
                    h_dest[:, :, 0:s], sig_io[:, :, s : 2 * s], tc_t[:, :, :s]
                )
                if root_sink is not None and S == 1:
                    # root state out on the sync queue, which is empty by now
                    # (no x loads remain): the collective trigger sees the
                    # completion sems as fast as the DMAs can land
                    gin = root_sink
                    for ch in range(2):
                        nc.sync.dma_start(
                            gin[ch * 128 : (ch + 1) * 128, 0:1], cs[:, ch, :]
                        )
                        # bf16->f32 cast DMA: gpsimd only
                        nc.gpsimd.dma_start(
                            gin[ch * 128 : (ch + 1) * 128, 1:2], h_dest[:, ch, 0:1]
                        )
                for ch in range(2):
                    nc.sync.dma_start(
                        out[ch, :, out_col0 : out_col0 + S], c_dest[:, ch, :S]
                    )
                    nc.gpsimd.dma_start(
                        outh[ch, :, out_col0 : out_col0 + S], h_dest[:, ch, :S]
                    )

            # ---- gather bounce buffers ----
            gin = dram.tile([256, 2], f32)
            # anti-DCE sink for the warm-up (overwritten by the root DMAs)
            nc.sync.dma_start(gin[0:128, :], jo[:])
            # Shared-output allgather: ncfw writes the gathered buffer once
            # into the shared scratchpad instead of bouncing per-rank.
            gout = nc.dram_tensor(
                "gout", [256 * NCORES, 2], f32, kind="Internal", addr_space="Shared"
            )

            # ---- sharded levels 15..3 ----
            pending = []
            for d in LEVELS:
                S = S_OF[d]
                par = d & 1
                h_child = c_child = f_child = None
                if d < DEPTH:
                    h_child = hbuf[1 - par][:, :, : 2 * S]
                    c_child = cbuf[1 - par][:, :, : 2 * S]
                    f_child = fbuf[1 - par][:, :, : 2 * S]
                if S > SMALL_S:
                    pending = do_level(
                        S,
                        OFF_OF[d],
                        OFF_OF[d],
                        h_child,
                        c_child,
                        f_child,
                        hbuf[par],
                        cbuf[par],
                        # parent uses the fpre path only while it is itself
                        # a big level (d-1 >= 11)
                        fbuf[par][:, :, :S] if d >= 12 else None,
                        pending=pending,
                    )
                else:
                    for th in pending:
                        th()
                    pending = []
                    do_small_level(
                        S,
                        OFF_OF[d],
                        OFF_OF[d],
                        h_child,
                        c_child,
                        hbuf[par],
                        cbuf[par],
                        root_sink=gin if d == SUB_DEPTH else None,
                    )

            # ---- allgather the 8 subtree roots ----
            nc.gpsimd.collective_compute(
                "AllGather",
                mybir.AluOpType.bypass,
                replica_groups=[list(range(NCORES))],
                ins=[gin.opt()],
                outs=[gout[:, :].opt()],
            )
            # load gathered roots feature-major: (p, ch, rank)
            g_ap = gout[:, :].rearrange("(r c p) t -> c p r t", p=128, c=2)
            c_top = ppool.tile([128, 2, 8], f32, tag="ctop")
            h_topf = ppool.tile([128, 2, 8], f32, tag="htopf")
            for ch in range(2):
                nc.sync.dma_start(c_top[:, ch, :], g_ap[ch, :, :, 0])
                nc.sync.dma_start(h_topf[:, ch, :], g_ap[ch, :, :, 1])
            h_top = ppool.tile([128, 2, 8], bf16, tag="htop")
            nc.vector.tensor_copy(h_top[:], h_topf[:])

            # ---- top levels 2..0 (computed redundantly on every core) ----
            prev_h, prev_c = h_top, c_top
            for d in (2, 1, 0):
                S = 2**d
                node0 = S - 1
                col0 = TOP_COL0 + node0
                h_d = ppool.tile([128, 2, S], bf16, tag=f"ht{d}", name=f"ht{d}")
                c_d = ppool.tile([128, 2, S], f32, tag=f"ct{d}", name=f"ct{d}")
                do_small_level(
                    S,
                    col0,
                    col0,
                    prev_h[:, :, : 2 * S],
                    prev_c[:, :, : 2 * S],
                    h_d,
                    c_d,
                )
                prev_h, prev_c = h_d, c_d

    nc.compile()
    return nc


def _get_program():
    if "nc" not in _CACHE:
        _CACHE["nc"] = _build_program()
    return _CACHE["nc"]


def _preprocess(inputs, W_ioux, b_ioux, W_iouh, b_iouh, W_fx, b_fx, W_fh, b_fh):
    """Build per-core input maps (numpy only)."""
    bf = ml_dtypes.bfloat16
    wx_cat = np.concatenate([W_ioux, W_fx], axis=0)  # [1024, 300]
    b_cat = np.concatenate([b_ioux + b_iouh, b_fx + b_fh], axis=0)  # [1024]
    # K padded to 384 with zero weight rows (keeps FWL on for every matmul)
    wx_full = np.zeros((K_PAD, 4 * MEM), np.float32)
    wx_full[:IN_DIM] = wx_cat.T
    wx_full[IN_DIM] = b_cat
    wx_np = np.ascontiguousarray(wx_full).astype(bf)
    wh_np = np.ascontiguousarray(W_iouh.T).astype(bf)  # [256, 768]
    wf_np = np.ascontiguousarray(W_fh.T).astype(bf)  # [256, 256]

    xT = np.zeros((K_PAD, N_NODES), np.float32)
    xT[:IN_DIM] = inputs.T
    xT[IN_DIM] = 1.0
    xT = xT.astype(bf)  # [384, 65535]

    in_maps = []
    for j in range(NCORES):
        segs = []
        for d in LEVELS:
            S = S_OF[d]
            g0 = (2**d - 1) + j * S
            segs.append(np.arange(g0, g0 + S))
        segs.append(np.arange(0, 7))
        cols = np.concatenate(segs)
        xcore = np.ascontiguousarray(xT[:, cols])
        in_maps.append({"xt": xcore, "wx": wx_np, "wh": wh_np, "wf": wf_np})
    return in_maps


def _postprocess(results):
    """Assemble [2, N, 256] from per-core outputs (c from `out` row 0, h from
    the bf16 `outh`, upcast on host)."""
    full = np.empty((2, N_NODES, MEM), np.float32)
    for j in range(NCORES):
        rc = results[j]["out"]  # [2(ch), 128, N_COLS] f32
        rh = results[j]["outh"].astype(np.float32)  # [2(ch), 128, N_COLS]
        r = np.stack([rc, rh])  # [2(c/h), 2, 128, N_COLS]
        for d in LEVELS:
            S = S_OF[d]
            g0 = (2**d - 1) + j * S
            off = OFF_OF[d]
            blk = r[:, :, :, off : off + S]  # [2,2,128,S]
            full[:, g0 : g0 + S, :] = blk.transpose(0, 3, 1, 2).reshape(2, S, MEM)
        if j == 0:
            r0 = r[:, :, :, TOP_COL0 : TOP_COL0 + 7]
            full[:, 0:7, :] = r0.transpose(0, 3, 1, 2).reshape(2, 7, MEM)
    return full


def kernel(**inputs):
    from concourse.bass_utils import run_bass_kernel_spmd

    nc = _get_program()
    inputs = {k: np.asarray(v) for k, v in inputs.items()}
    in_maps = _preprocess(**inputs)
    res = run_bass_kernel_spmd(nc, in_maps, core_ids=list(range(NCORES)))
    _CACHE["last_result"] = res
    return _postprocess(res.results)

